# revision 19
# baseline (speedup 1.0000x reference)
"""Trainium2 Bass kernel for the dual-domain (coded/PAN) proximal update.

Math (per sample b, with w* = scalar weights from mu/mu_c/mu_p):
  a_c   = z + (mu/mu_c) * xp_k_1                 (elementwise)
  S1    = sum_c phi_c * a_c                      (channel reduction)
  D     = (mu + mu_c) + sum_c phi_c^2            (channel reduction)
  beta  = (yc - wc * S1) / D                     (per h,w)
  xc    = wc * a_c + phi_c * beta                (beta broadcast over c)
and symmetrically for the p-domain with z replaced by the per-channel
shifted z (z_sb[c,h,w] = z[c,h,w+2c], which never wraps since
w + 2*c <= 255 + 54 < 310 -- so it is a pure strided view of z with
channel stride H*W + 2).

Sharding: pure data-parallel, batch element b -> NeuronCore b.

On-chip layout: partition dim packs (hs=4 h-rows) x (28 channels) = 112
partitions; free dim packs (J=8 h-blocks) x W.  Channel reductions and the
beta broadcast run on the TensorEngine as ones/selector matmuls (fp32r),
elementwise work on Vector/Scalar engines.
"""

import numpy as np

B, NC, H, WC, WP = 8, 28, 256, 310, 256
STEP = 2
HS = 4          # h-rows packed into the partition dim
J = 8           # h-blocks packed into the free dim per round
ROWS = HS * J   # h-rows per round (32)
R = H // ROWS   # rounds per domain (8)
P = HS * NC     # used partitions (112)

F32 = None  # set lazily (mybir.dt.float32)

# module-level knobs / results (test.py pokes these; harness uses defaults)
TRACE = False
LAST_RESULTS = None

_PROGRAM_CACHE = {}


def _weights_np(wneg, dconst):
    """Selector matrices for the channel-reduce / broadcast matmuls.

    PE constraints force matmul outputs to base partition 0, so each j-block
    gets its own selector with nonzero entries only in its 4 output columns;
    the 8 reduce-matmuls accumulate into one [32, W] PSUM tile.

    W1 [112, 8*32]: block j: out[4j+hs, w] += wneg * sum_c in[(hs,c), (j,w)]
    W2 [113, 8*32]: like W1 with weight 1.0; row 112 (constant-1.0 row of the
                    squares tile) adds dconst to the 4 columns of block j.
    Wb [32, 8*112]: block j: out[(hs,c), w] = beta[4j+hs, w] (broadcast).
    """
    w1 = np.zeros((P, J, ROWS), np.float32)
    w2 = np.zeros((P, J, ROWS), np.float32)
    wb = np.zeros((ROWS, J, P), np.float32)
    for p in range(P):
        hs = p // NC
        for j in range(J):
            # row within the round: q = hs*J + j (hs picks an 8-row band)
            w1[p, j, hs * J + j] = wneg
            w2[p, j, hs * J + j] = 1.0
            wb[hs * J + j, j, p] = 1.0
    return w1.reshape(P, -1), w2.reshape(P, -1), wb.reshape(ROWS, -1)


def _build(mu_, mu_c_, mu_p_):
    import concourse.bacc as bacc
    import concourse.bass as bass
    import concourse.mybir as mybir
    from concourse.tile import TileContext

    f32 = mybir.dt.float32
    f32r = mybir.dt.float32r
    Alu = mybir.AluOpType

    wc = mu_c_ / (mu_ + mu_c_)
    rc = mu_ / mu_c_          # wz/wc
    dc0 = mu_ + mu_c_
    wp = mu_p_ / (mu_ + mu_p_)
    rp = mu_ / mu_p_          # wz2/wp
    dp0 = mu_ + mu_p_

    nc = bacc.Bacc("TRN2", target_bir_lowering=False, debug=False)

    def inline_f32r(data, name):
        # like nc.inline_tensor but with dtype float32r so the DMA'd weights
        # carry the "rounded for FP32r matmul" tag the BIR verifier requires
        import base64
        import io

        data = np.ascontiguousarray(data, np.float32)
        mls = nc._tensor(name, list(data.shape), f32r, kind="Const", type="DRAM")
        buf = io.BytesIO()
        np.save(buf, data, allow_pickle=False)
        mls.file = f"{name}.npy"
        mls.ant_data = base64.standard_b64encode(buf.getvalue()).decode()
        return bass.DRamTensorHandle(name, list(data.shape), f32r)

    z_h = nc.dram_tensor("z", [NC, H, WC], f32, kind="ExternalInput")
    phic_h = nc.dram_tensor("phi_c", [NC, H, WC], f32, kind="ExternalInput")
    xp1_h = nc.dram_tensor("xp_k_1", [NC, H, WC], f32, kind="ExternalInput")
    yc_h = nc.dram_tensor("yc", [H, WC], f32, kind="ExternalInput")
    phip_h = nc.dram_tensor("phi_p", [NC, H, WP], f32, kind="ExternalInput")
    xc1_h = nc.dram_tensor("xc_k_1", [NC, H, WP], f32, kind="ExternalInput")
    yp_h = nc.dram_tensor("yp", [H, WP], f32, kind="ExternalInput")
    xc_h = nc.dram_tensor("xc", [NC, H, WC], f32, kind="ExternalOutput")
    xp_h = nc.dram_tensor("xp", [NC, H, WP], f32, kind="ExternalOutput")

    w1c_np, w2c_np, wb_np = _weights_np(-wc, dc0)
    w1p_np, w2p_np, _ = _weights_np(-wp, dp0)
    assert wb_np.shape == (ROWS, J * P)

    # all selector weights in one [128, 5*256... ] wall: one DMA + one PE
    # warm-up matmul leaves every real matmul with <=1 sync wait (walrus
    # rejects fused-LDW matmuls with more).
    wall_np = np.zeros((128, 4 * J * ROWS + J * P), np.float32)
    wall_np[:P, 0 * J * ROWS : 1 * J * ROWS] = w1c_np
    wall_np[:P, 1 * J * ROWS : 2 * J * ROWS] = w2c_np
    wall_np[:P, 2 * J * ROWS : 3 * J * ROWS] = w1p_np
    wall_np[:P, 3 * J * ROWS : 4 * J * ROWS] = w2p_np
    wall_np[:ROWS, 4 * J * ROWS :] = wb_np
    wall_d = inline_f32r(wall_np, "wall_d")

    def dram_ap(handle, offset, ap):
        base = handle[:, :] if len(handle.shape) == 2 else handle[:, :, :]
        return bass.AP(tensor=base.tensor, offset=offset, ap=[list(d) for d in ap])

    with TileContext(nc) as tc:
        with (
            tc.tile_pool(name="bigs", bufs=2) as bigs,
            tc.tile_pool(name="smalls", bufs=2) as smalls,
            tc.tile_pool(name="singles", bufs=1) as singles,
            tc.tile_pool(name="psum", bufs=2, space="PSUM") as psum,
        ):
            wall = singles.tile([128, 4 * J * ROWS + J * P], f32r)
            nc.sync.dma_start(out=wall, in_=wall_d[:, :])
            w1c = wall[:P, 0 * J * ROWS : 1 * J * ROWS]
            w2c = wall[:P, 1 * J * ROWS : 2 * J * ROWS]
            w1p = wall[:P, 2 * J * ROWS : 3 * J * ROWS]
            w2p = wall[:P, 3 * J * ROWS : 4 * J * ROWS]
            wb = wall[:ROWS, 4 * J * ROWS :]
            warm = psum.tile([2, 2], f32, tag="warm", bufs=1)
            nc.tensor.matmul(warm, wall[:, 0:2], wall[:, 0:2], start=True, stop=True)

            # persistent squares tiles (parity double-buffer)
            sq0 = singles.tile([P, J, WC], f32r)
            sq1 = singles.tile([P, J, WC], f32r)
            sq_tiles = [sq0, sq1]

            def round_(dom, r, parity):
                if dom == "c":
                    W = WC
                    src_z, src_phi, src_prev, src_y = z_h, phic_h, xp1_h, yc_h
                    dst = xc_h
                    w1, w2 = w1c, w2c
                    wmain, raux = wc, rc
                else:
                    W = WP
                    src_z, src_phi, src_prev, src_y = z_h, phip_h, xc1_h, yp_h
                    dst = xp_h
                    w1, w2 = w1p, w2p
                    wmain, raux = wp, rp

                h0 = r * ROWS
                # partition = (hs, c) with hs = row//J (8-row bands), so the
                # per-partition free run is J*W contiguous DRAM elements and
                # the DMA balances to 3 dims.
                oap = [[J * W, HS], [H * W, NC], [1, J * W]]
                ooff = h0 * W

                zt = bigs.tile([P, J, W], f32, tag="z", name=f"z_{dom}{r}")
                if dom == "c":
                    zap = [[J * WC, HS], [H * WC, NC], [1, J * WC]]
                    nc.sync.dma_start(out=zt, in_=dram_ap(src_z, h0 * WC, zap))
                else:
                    # shifted view: w-runs of WP at row stride WC don't merge
                    # with the j dim -> one DMA per hs band (3 dims each)
                    for hs in range(HS):
                        nc.sync.dma_start(
                            out=zt[NC * hs : NC * (hs + 1), :, :],
                            in_=dram_ap(
                                src_z,
                                (h0 + hs * J) * WC,
                                [[H * WC + STEP, NC], [WC, J], [1, WP]],
                            ),
                        )
                pt = bigs.tile([P, J, W], f32, tag="phi", name=f"phi_{dom}{r}")
                nc.sync.dma_start(out=pt, in_=dram_ap(src_phi, ooff, oap))
                xt = bigs.tile([P, J, W], f32, tag="prev", name=f"prev_{dom}{r}")
                nc.sync.dma_start(out=xt, in_=dram_ap(src_prev, ooff, oap))
                yt = smalls.tile([ROWS, W], f32, tag="y", name=f"y_{dom}{r}")
                nc.sync.dma_start(out=yt, in_=src_y[h0 : h0 + ROWS, :])

                # a = z + raux * prev
                at = bigs.tile([P, J, W], f32, tag="a", name=f"a_{dom}{r}")
                nc.vector.scalar_tensor_tensor(
                    out=at, in0=xt, scalar=raux, in1=zt, op0=Alu.mult, op1=Alu.add
                )

                # squares on the scalar engine (row P stays 1.0)
                sq = sq_tiles[parity]
                nc.scalar.square(out=sq[:, :, :W], in_=pt)

                # products phi*a for the S1 reduction
                p1 = bigs.tile([P, J, W], f32r, tag="p1", name=f"p1_{dom}{r}")
                nc.vector.tensor_tensor(out=p1, in0=pt, in1=at, op=Alu.mult)

                # channel reductions on PE: S1 = -w * sum_c phi*a ; D = dofs + sum_c phi^2
                # each j-block has its own selector columns; the 8 matmuls
                # accumulate into one [32, W] PSUM tile at base partition 0.
                s1 = psum.tile([ROWS, W], f32, tag="s1", name=f"s1_{dom}{r}")
                s2 = psum.tile([ROWS, W], f32, tag="s2", name=f"s2_{dom}{r}")
                for j in range(J):
                    nc.tensor.matmul(
                        s1,
                        w1[:, ROWS * j : ROWS * (j + 1)],
                        p1[:, j, :],
                        start=(j == 0),
                        stop=(j == J - 1),
                    )
                for j in range(J):
                    nc.tensor.matmul(
                        s2,
                        w2[:, ROWS * j : ROWS * (j + 1)],
                        sq[:, j, :W],
                        start=(j == 0),
                        stop=(j == J - 1),
                    )

                # beta = (y + S1) / (S2 + dofs)
                dofs = dc0 if dom == "c" else dp0
                tt = smalls.tile([ROWS, W], f32, tag="tt", name=f"tt_{dom}{r}")
                nc.vector.tensor_tensor(out=tt, in0=yt, in1=s1, op=Alu.add)
                dd = smalls.tile([ROWS, W], f32, tag="dd", name=f"dd_{dom}{r}")
                nc.vector.tensor_scalar_add(dd, s2, dofs)
                rcp = smalls.tile([ROWS, W], f32, tag="rcp", name=f"rcp_{dom}{r}")
                nc.vector.reciprocal_approx_fast(out=rcp, in_=dd)
                bt = smalls.tile([ROWS, W], f32r, tag="bt", name=f"bt_{dom}{r}")
                nc.vector.tensor_tensor(out=bt, in0=tt, in1=rcp, op=Alu.mult)

                # broadcast beta over channels (PE) and apply: out = w*a + phi*beta
                t2 = bigs.tile([P, J, W], f32, tag="t2", name=f"t2_{dom}{r}")
                for j in range(J):
                    bj = psum.tile([P, W], f32, tag="bj", bufs=3, name=f"bj_{dom}{r}_{j}")
                    nc.tensor.matmul(
                        bj,
                        wb[:, P * j : P * (j + 1)],
                        bt[:, :],
                        start=True,
                        stop=True,
                    )
                    nc.vector.tensor_tensor(
                        out=t2[:, j, :], in0=pt[:, j, :], in1=bj, op=Alu.mult
                    )

                ot = bigs.tile([P, J, W], f32, tag="out", name=f"out_{dom}{r}")
                nc.vector.scalar_tensor_tensor(
                    out=ot, in0=at, scalar=wmain, in1=t2, op0=Alu.mult, op1=Alu.add
                )
                nc.sync.dma_start(out=dram_ap(dst, ooff, oap), in_=ot)

            i = 0
            for dom in ("c", "p"):
                for r in range(R):
                    round_(dom, r, i % 2)
                    i += 1

    nc.compile()
    return nc


def kernel(z, yc, phi_c, yp, phi_p, xc_k_1, xp_k_1, mu, mu_c, mu_p):
    global LAST_RESULTS
    from concourse.bass_utils import run_bass_kernel_spmd

    mu_ = float(np.asarray(mu).reshape(-1)[0])
    mu_c_ = float(np.asarray(mu_c).reshape(-1)[0])
    mu_p_ = float(np.asarray(mu_p).reshape(-1)[0])

    key = (mu_, mu_c_, mu_p_)
    nc = _PROGRAM_CACHE.get(key)
    if nc is None:
        nc = _build(mu_, mu_c_, mu_p_)
        _PROGRAM_CACHE[key] = nc

    z = np.ascontiguousarray(np.asarray(z), np.float32)
    yc = np.ascontiguousarray(np.asarray(yc), np.float32)
    phi_c = np.ascontiguousarray(np.asarray(phi_c), np.float32)
    yp = np.ascontiguousarray(np.asarray(yp), np.float32)
    phi_p = np.ascontiguousarray(np.asarray(phi_p), np.float32)
    xc_k_1 = np.ascontiguousarray(np.asarray(xc_k_1), np.float32)
    xp_k_1 = np.ascontiguousarray(np.asarray(xp_k_1), np.float32)

    in_maps = [
        {
            "z": z[b],
            "phi_c": phi_c[b],
            "xp_k_1": xp_k_1[b],
            "yc": yc[b],
            "phi_p": phi_p[b],
            "xc_k_1": xc_k_1[b],
            "yp": yp[b],
        }
        for b in range(B)
    ]

    res = run_bass_kernel_spmd(nc, in_maps, core_ids=list(range(B)), trace=TRACE)
    LAST_RESULTS = res
    xc = np.stack([res.results[b]["xc"] for b in range(B)])
    xp = np.stack([res.results[b]["xp"] for b in range(B)])
    return xc, xp


# revision 20
# speedup vs baseline: 1.1019x; 1.1019x over previous
"""Trainium2 Bass kernel for the dual-domain (coded/PAN) proximal update.

Math (per sample b, with w* = scalar weights from mu/mu_c/mu_p):
  a_c   = z + (mu/mu_c) * xp_k_1                 (elementwise)
  S1    = sum_c phi_c * a_c                      (channel reduction)
  D     = (mu + mu_c) + sum_c phi_c^2            (channel reduction)
  beta  = (yc - wc * S1) / D                     (per h,w)
  xc    = wc * a_c + phi_c * beta                (beta broadcast over c)
and symmetrically for the p-domain with z replaced by the per-channel
shifted z (z_sb[c,h,w] = z[c,h,w+2c], which never wraps since
w + 2*c <= 255 + 54 < 310 -- so it is a pure strided view of z with
channel stride H*W + 2).

Sharding: pure data-parallel, batch element b -> NeuronCore b.

On-chip layout: partition dim packs (hs=4 h-rows) x (28 channels) = 112
partitions; free dim packs (J=8 h-blocks) x W.  Channel reductions and the
beta broadcast run on the TensorEngine as ones/selector matmuls (fp32r),
elementwise work on Vector/Scalar engines.
"""

import numpy as np

B, NC, H, WC, WP = 8, 28, 256, 310, 256
STEP = 2
HS = 4          # h-rows packed into the partition dim
J = 8           # h-blocks packed into the free dim per round
ROWS = HS * J   # h-rows per round (32)
R = H // ROWS   # rounds per domain (8)
P = HS * NC     # used partitions (112)

F32 = None  # set lazily (mybir.dt.float32)

# module-level knobs / results (test.py pokes these; harness uses defaults)
TRACE = False
LAST_RESULTS = None

_PROGRAM_CACHE = {}


def _weights_np(wneg, dconst):
    """Selector matrices for the channel-reduce / broadcast matmuls.

    PE constraints force matmul outputs to base partition 0, so each j-block
    gets its own selector with nonzero entries only in its 4 output columns;
    the 8 reduce-matmuls accumulate into one [32, W] PSUM tile.

    W1 [112, 8*32]: block j: out[4j+hs, w] += wneg * sum_c in[(hs,c), (j,w)]
    W2 [113, 8*32]: like W1 with weight 1.0; row 112 (constant-1.0 row of the
                    squares tile) adds dconst to the 4 columns of block j.
    Wb [32, 8*112]: block j: out[(hs,c), w] = beta[4j+hs, w] (broadcast).
    """
    w1 = np.zeros((P, J, ROWS), np.float32)
    w2 = np.zeros((P, J, ROWS), np.float32)
    wb = np.zeros((ROWS, J, P), np.float32)
    for p in range(P):
        hs = p // NC
        for j in range(J):
            # row within the round: q = hs*J + j (hs picks an 8-row band)
            w1[p, j, hs * J + j] = wneg
            w2[p, j, hs * J + j] = 1.0
            wb[hs * J + j, j, p] = 1.0
    return w1.reshape(P, -1), w2.reshape(P, -1), wb.reshape(ROWS, -1)


def _build(mu_, mu_c_, mu_p_):
    import concourse.bacc as bacc
    import concourse.bass as bass
    import concourse.mybir as mybir
    from concourse.tile import TileContext

    f32 = mybir.dt.float32
    f32r = mybir.dt.float32r
    Alu = mybir.AluOpType

    wc = mu_c_ / (mu_ + mu_c_)
    rc = mu_ / mu_c_          # wz/wc
    dc0 = mu_ + mu_c_
    wp = mu_p_ / (mu_ + mu_p_)
    rp = mu_ / mu_p_          # wz2/wp
    dp0 = mu_ + mu_p_

    nc = bacc.Bacc("TRN2", target_bir_lowering=False, debug=False)

    def inline_f32r(data, name):
        # like nc.inline_tensor but with dtype float32r so the DMA'd weights
        # carry the "rounded for FP32r matmul" tag the BIR verifier requires
        import base64
        import io

        data = np.ascontiguousarray(data, np.float32)
        mls = nc._tensor(name, list(data.shape), f32r, kind="Const", type="DRAM")
        buf = io.BytesIO()
        np.save(buf, data, allow_pickle=False)
        mls.file = f"{name}.npy"
        mls.ant_data = base64.standard_b64encode(buf.getvalue()).decode()
        return bass.DRamTensorHandle(name, list(data.shape), f32r)

    z_h = nc.dram_tensor("z", [NC, H, WC], f32, kind="ExternalInput")
    phic_h = nc.dram_tensor("phi_c", [NC, H, WC], f32, kind="ExternalInput")
    xp1_h = nc.dram_tensor("xp_k_1", [NC, H, WC], f32, kind="ExternalInput")
    yc_h = nc.dram_tensor("yc", [H, WC], f32, kind="ExternalInput")
    phip_h = nc.dram_tensor("phi_p", [NC, H, WP], f32, kind="ExternalInput")
    xc1_h = nc.dram_tensor("xc_k_1", [NC, H, WP], f32, kind="ExternalInput")
    yp_h = nc.dram_tensor("yp", [H, WP], f32, kind="ExternalInput")
    xc_h = nc.dram_tensor("xc", [NC, H, WC], f32, kind="ExternalOutput")
    xp_h = nc.dram_tensor("xp", [NC, H, WP], f32, kind="ExternalOutput")

    w1c_np, w2c_np, wb_np = _weights_np(-wc, dc0)
    w1p_np, w2p_np, _ = _weights_np(-wp, dp0)
    assert wb_np.shape == (ROWS, J * P)

    # all selector weights in one [128, 5*256... ] wall: one DMA + one PE
    # warm-up matmul leaves every real matmul with <=1 sync wait (walrus
    # rejects fused-LDW matmuls with more).
    wall_np = np.zeros((128, 4 * J * ROWS + J * P), np.float32)
    wall_np[:P, 0 * J * ROWS : 1 * J * ROWS] = w1c_np
    wall_np[:P, 1 * J * ROWS : 2 * J * ROWS] = w2c_np
    wall_np[:P, 2 * J * ROWS : 3 * J * ROWS] = w1p_np
    wall_np[:P, 3 * J * ROWS : 4 * J * ROWS] = w2p_np
    wall_np[:ROWS, 4 * J * ROWS :] = wb_np
    wall_d = inline_f32r(wall_np, "wall_d")

    def dram_ap(handle, offset, ap):
        base = handle[:, :] if len(handle.shape) == 2 else handle[:, :, :]
        return bass.AP(tensor=base.tensor, offset=offset, ap=[list(d) for d in ap])

    with TileContext(nc) as tc:
        with (
            tc.tile_pool(name="bigs", bufs=2) as bigs,
            tc.tile_pool(name="smalls", bufs=2) as smalls,
            tc.tile_pool(name="singles", bufs=1) as singles,
            tc.tile_pool(name="psum", bufs=2, space="PSUM") as psum,
        ):
            wall = singles.tile([128, 4 * J * ROWS + J * P], f32r)
            nc.sync.dma_start(out=wall, in_=wall_d[:, :])
            w1c = wall[:P, 0 * J * ROWS : 1 * J * ROWS]
            w2c = wall[:P, 1 * J * ROWS : 2 * J * ROWS]
            w1p = wall[:P, 2 * J * ROWS : 3 * J * ROWS]
            w2p = wall[:P, 3 * J * ROWS : 4 * J * ROWS]
            wb = wall[:ROWS, 4 * J * ROWS :]
            warm = psum.tile([2, 2], f32, tag="s1", bufs=2)
            nc.tensor.matmul(warm, wall[:, 0:2], wall[:, 0:2], start=True, stop=True)

            # persistent squares tiles (parity double-buffer)
            sq0 = singles.tile([P, J, WC], f32r)
            sq1 = singles.tile([P, J, WC], f32r)
            sq_tiles = [sq0, sq1]

            def round_(dom, r, parity):
                if dom == "c":
                    W = WC
                    src_z, src_phi, src_prev, src_y = z_h, phic_h, xp1_h, yc_h
                    dst = xc_h
                    w1, w2 = w1c, w2c
                    wmain, raux = wc, rc
                else:
                    W = WP
                    src_z, src_phi, src_prev, src_y = z_h, phip_h, xc1_h, yp_h
                    dst = xp_h
                    w1, w2 = w1p, w2p
                    wmain, raux = wp, rp

                h0 = r * ROWS
                # partition = (hs, c) with hs = row//J (8-row bands), so the
                # per-partition free run is J*W contiguous DRAM elements and
                # the DMA balances to 3 dims.
                oap = [[J * W, HS], [H * W, NC], [1, J * W]]
                ooff = h0 * W

                zt = bigs.tile([P, J, W], f32, tag="z", name=f"z_{dom}{r}")
                if dom == "c":
                    zap = [[J * WC, HS], [H * WC, NC], [1, J * WC]]
                    nc.sync.dma_start(out=zt, in_=dram_ap(src_z, h0 * WC, zap))
                else:
                    # shifted view: w-runs of WP at row stride WC don't merge
                    # with the j dim -> one DMA per hs band (3 dims each)
                    for hs in range(HS):
                        nc.gpsimd.dma_start(
                            out=zt[NC * hs : NC * (hs + 1), :, :],
                            in_=dram_ap(
                                src_z,
                                (h0 + hs * J) * WC,
                                [[H * WC + STEP, NC], [WC, J], [1, WP]],
                            ),
                        )
                pt = bigs.tile([P, J, W], f32, tag="phi", name=f"phi_{dom}{r}")
                nc.sync.dma_start(out=pt, in_=dram_ap(src_phi, ooff, oap))
                xt = bigs.tile([P, J, W], f32, tag="prev", name=f"prev_{dom}{r}")
                nc.scalar.dma_start(out=xt, in_=dram_ap(src_prev, ooff, oap))
                yt = smalls.tile([ROWS, W], f32, tag="y", name=f"y_{dom}{r}")
                nc.gpsimd.dma_start(out=yt, in_=src_y[h0 : h0 + ROWS, :])

                # a = z + raux * prev
                at = bigs.tile([P, J, W], f32, tag="a", name=f"a_{dom}{r}")
                nc.vector.scalar_tensor_tensor(
                    out=at, in0=xt, scalar=raux, in1=zt, op0=Alu.mult, op1=Alu.add
                )

                # squares on the scalar engine (row P stays 1.0)
                sq = sq_tiles[parity]
                nc.scalar.square(out=sq[:, :, :W], in_=pt)

                # products phi*a for the S1 reduction
                p1 = bigs.tile([P, J, W], f32r, tag="p1", name=f"p1_{dom}{r}")
                nc.vector.tensor_tensor(out=p1, in0=pt, in1=at, op=Alu.mult)

                # channel reductions on PE: S1 = -w * sum_c phi*a ; D = dofs + sum_c phi^2
                # each j-block has its own selector columns; the 8 matmuls
                # accumulate into one [32, W] PSUM tile at base partition 0.
                s1 = psum.tile([ROWS, W], f32, tag="s1", name=f"s1_{dom}{r}")
                s2 = psum.tile([ROWS, W], f32, tag="s2", name=f"s2_{dom}{r}")
                for j in range(J):
                    nc.tensor.matmul(
                        s1,
                        w1[:, ROWS * j : ROWS * (j + 1)],
                        p1[:, j, :],
                        start=(j == 0),
                        stop=(j == J - 1),
                    )
                for j in range(J):
                    nc.tensor.matmul(
                        s2,
                        w2[:, ROWS * j : ROWS * (j + 1)],
                        sq[:, j, :W],
                        start=(j == 0),
                        stop=(j == J - 1),
                    )

                # beta = (y + S1) / (S2 + dofs)
                dofs = dc0 if dom == "c" else dp0
                tt = smalls.tile([ROWS, W], f32, tag="tt", name=f"tt_{dom}{r}")
                nc.vector.tensor_tensor(out=tt, in0=yt, in1=s1, op=Alu.add)
                dd = smalls.tile([ROWS, W], f32, tag="dd", name=f"dd_{dom}{r}")
                nc.scalar.activation(
                    out=dd, in_=s2, func=mybir.ActivationFunctionType.Copy, bias=dofs
                )
                rcp = smalls.tile([ROWS, W], f32, tag="rcp", name=f"rcp_{dom}{r}")
                nc.vector.reciprocal_approx_fast(out=rcp, in_=dd)
                bt = smalls.tile([ROWS, W], f32r, tag="bt", name=f"bt_{dom}{r}")
                nc.vector.tensor_tensor(out=bt, in0=tt, in1=rcp, op=Alu.mult)

                # broadcast beta over channels (PE) and apply: out = w*a + phi*beta
                t2 = bigs.tile([P, J, W], f32, tag="t2", name=f"t2_{dom}{r}")
                for g in range(J // 2):
                    bj = psum.tile(
                        [P, 2, 512], f32, tag="bj", bufs=2, name=f"bj_{dom}{r}_{g}"
                    )
                    for jj in range(2):
                        nc.tensor.matmul(
                            bj[:, jj, :W],
                            wb[:, P * (2 * g + jj) : P * (2 * g + jj + 1)],
                            bt[:, :],
                            start=True,
                            stop=True,
                        )
                    nc.vector.tensor_tensor(
                        out=t2[:, 2 * g : 2 * g + 2, :],
                        in0=pt[:, 2 * g : 2 * g + 2, :],
                        in1=bj[:, :, :W],
                        op=Alu.mult,
                    )

                ot = bigs.tile([P, J, W], f32, tag="out", name=f"out_{dom}{r}")
                nc.vector.scalar_tensor_tensor(
                    out=ot, in0=at, scalar=wmain, in1=t2, op0=Alu.mult, op1=Alu.add
                )
                nc.gpsimd.dma_start(out=dram_ap(dst, ooff, oap), in_=ot)

            i = 0
            for dom in ("c", "p"):
                for r in range(R):
                    round_(dom, r, i % 2)
                    i += 1

    nc.compile()
    return nc


def kernel(z, yc, phi_c, yp, phi_p, xc_k_1, xp_k_1, mu, mu_c, mu_p):
    global LAST_RESULTS
    from concourse.bass_utils import run_bass_kernel_spmd

    mu_ = float(np.asarray(mu).reshape(-1)[0])
    mu_c_ = float(np.asarray(mu_c).reshape(-1)[0])
    mu_p_ = float(np.asarray(mu_p).reshape(-1)[0])

    key = (mu_, mu_c_, mu_p_)
    nc = _PROGRAM_CACHE.get(key)
    if nc is None:
        nc = _build(mu_, mu_c_, mu_p_)
        _PROGRAM_CACHE[key] = nc

    z = np.ascontiguousarray(np.asarray(z), np.float32)
    yc = np.ascontiguousarray(np.asarray(yc), np.float32)
    phi_c = np.ascontiguousarray(np.asarray(phi_c), np.float32)
    yp = np.ascontiguousarray(np.asarray(yp), np.float32)
    phi_p = np.ascontiguousarray(np.asarray(phi_p), np.float32)
    xc_k_1 = np.ascontiguousarray(np.asarray(xc_k_1), np.float32)
    xp_k_1 = np.ascontiguousarray(np.asarray(xp_k_1), np.float32)

    in_maps = [
        {
            "z": z[b],
            "phi_c": phi_c[b],
            "xp_k_1": xp_k_1[b],
            "yc": yc[b],
            "phi_p": phi_p[b],
            "xc_k_1": xc_k_1[b],
            "yp": yp[b],
        }
        for b in range(B)
    ]

    res = run_bass_kernel_spmd(nc, in_maps, core_ids=list(range(B)), trace=TRACE)
    LAST_RESULTS = res
    xc = np.stack([res.results[b]["xc"] for b in range(B)])
    xp = np.stack([res.results[b]["xp"] for b in range(B)])
    return xc, xp


# revision 22
# speedup vs baseline: 1.8712x; 1.6981x over previous
"""Trainium2 Bass kernel for the dual-domain (coded/PAN) proximal update.

Math (per sample b, with w* = scalar weights from mu/mu_c/mu_p):
  a_c   = z + (mu/mu_c) * xp_k_1                 (elementwise)
  S1    = sum_c phi_c * a_c                      (channel reduction)
  D     = (mu + mu_c) + sum_c phi_c^2            (channel reduction)
  beta  = (yc - wc * S1) / D                     (per h,w)
  xc    = wc * a_c + phi_c * beta                (beta broadcast over c)
and symmetrically for the p-domain with z replaced by the per-channel
shifted z (z_sb[c,h,w] = z[c,h,w+2c], which never wraps since
w + 2*c <= 255 + 54 < 310 -- so it is a pure strided view of z with
channel stride H*W + 2).

Sharding: pure data-parallel, batch element b -> NeuronCore b.

On-chip layout: partition dim packs (hs=4 h-rows) x (28 channels) = 112
partitions; free dim packs (J=8 h-blocks) x W.  Channel reductions and the
beta broadcast run on the TensorEngine as ones/selector matmuls (fp32r),
elementwise work on Vector/Scalar engines.
"""

import numpy as np

B, NC, H, WC, WP = 8, 28, 256, 310, 256
STEP = 2
HS = 4          # h-rows packed into the partition dim
J = 8           # h-blocks packed into the free dim per round
ROWS = HS * J   # h-rows per round (32)
R = H // ROWS   # rounds per domain (8)
P = HS * NC     # used partitions (112)

F32 = None  # set lazily (mybir.dt.float32)

# module-level knobs / results (test.py pokes these; harness uses defaults)
TRACE = False
LAST_RESULTS = None

_PROGRAM_CACHE = {}


def _weights_np(wneg, dconst):
    """Selector matrices for the channel-reduce / broadcast matmuls.

    PE constraints force matmul outputs to base partition 0, so each j-block
    gets its own selector with nonzero entries only in its 4 output columns;
    the 8 reduce-matmuls accumulate into one [32, W] PSUM tile.

    W1 [112, 8*32]: block j: out[4j+hs, w] += wneg * sum_c in[(hs,c), (j,w)]
    W2 [113, 8*32]: like W1 with weight 1.0; row 112 (constant-1.0 row of the
                    squares tile) adds dconst to the 4 columns of block j.
    Wb [32, 8*112]: block j: out[(hs,c), w] = beta[4j+hs, w] (broadcast).
    """
    w1 = np.zeros((P, J, ROWS), np.float32)
    w2 = np.zeros((P, J, ROWS), np.float32)
    wb = np.zeros((ROWS, J, P), np.float32)
    for p in range(P):
        hs = p % HS  # c-major partition packing: p = c*HS + hs
        for j in range(J):
            # row within the round: q = hs*J + j (hs picks an 8-row band)
            w1[p, j, hs * J + j] = wneg
            w2[p, j, hs * J + j] = 1.0
            wb[hs * J + j, j, p] = 1.0
    return w1.reshape(P, -1), w2.reshape(P, -1), wb.reshape(ROWS, -1)


def _build(mu_, mu_c_, mu_p_):
    import concourse.bacc as bacc
    import concourse.bass as bass
    import concourse.mybir as mybir
    from concourse.tile import TileContext

    f32 = mybir.dt.float32
    f32r = mybir.dt.float32r
    Alu = mybir.AluOpType

    wc = mu_c_ / (mu_ + mu_c_)
    rc = mu_ / mu_c_          # wz/wc
    dc0 = mu_ + mu_c_
    wp = mu_p_ / (mu_ + mu_p_)
    rp = mu_ / mu_p_          # wz2/wp
    dp0 = mu_ + mu_p_

    nc = bacc.Bacc("TRN2", target_bir_lowering=False, debug=False)

    def inline_f32r(data, name):
        # like nc.inline_tensor but with dtype float32r so the DMA'd weights
        # carry the "rounded for FP32r matmul" tag the BIR verifier requires
        import base64
        import io

        data = np.ascontiguousarray(data, np.float32)
        mls = nc._tensor(name, list(data.shape), f32r, kind="Const", type="DRAM")
        buf = io.BytesIO()
        np.save(buf, data, allow_pickle=False)
        mls.file = f"{name}.npy"
        mls.ant_data = base64.standard_b64encode(buf.getvalue()).decode()
        return bass.DRamTensorHandle(name, list(data.shape), f32r)

    z_h = nc.dram_tensor("z", [NC, H, WC], f32, kind="ExternalInput")
    phic_h = nc.dram_tensor("phi_c", [NC, H, WC], f32, kind="ExternalInput")
    xp1_h = nc.dram_tensor("xp_k_1", [NC, H, WC], f32, kind="ExternalInput")
    yc_h = nc.dram_tensor("yc", [H, WC], f32, kind="ExternalInput")
    phip_h = nc.dram_tensor("phi_p", [NC, H, WP], f32, kind="ExternalInput")
    xc1_h = nc.dram_tensor("xc_k_1", [NC, H, WP], f32, kind="ExternalInput")
    yp_h = nc.dram_tensor("yp", [H, WP], f32, kind="ExternalInput")
    xc_h = nc.dram_tensor("xc", [NC, H, WC], f32, kind="ExternalOutput")
    xp_h = nc.dram_tensor("xp", [NC, H, WP], f32, kind="ExternalOutput")

    w1c_np, w2c_np, wb_np = _weights_np(-wc, dc0)
    w1p_np, w2p_np, _ = _weights_np(-wp, dp0)
    assert wb_np.shape == (ROWS, J * P)

    # all selector weights in one [128, 5*256... ] wall: one DMA + one PE
    # warm-up matmul leaves every real matmul with <=1 sync wait (walrus
    # rejects fused-LDW matmuls with more).
    wall_np = np.zeros((128, 4 * J * ROWS + J * P), np.float32)
    wall_np[:P, 0 * J * ROWS : 1 * J * ROWS] = w1c_np
    wall_np[:P, 1 * J * ROWS : 2 * J * ROWS] = w2c_np
    wall_np[:P, 2 * J * ROWS : 3 * J * ROWS] = w1p_np
    wall_np[:P, 3 * J * ROWS : 4 * J * ROWS] = w2p_np
    wall_np[:ROWS, 4 * J * ROWS :] = wb_np
    wall_d = inline_f32r(wall_np, "wall_d")

    def dram_ap(handle, offset, ap):
        base = handle[:, :] if len(handle.shape) == 2 else handle[:, :, :]
        return bass.AP(tensor=base.tensor, offset=offset, ap=[list(d) for d in ap])

    with TileContext(nc) as tc:
        with (
            tc.tile_pool(name="bigs", bufs=2) as bigs,
            tc.tile_pool(name="smalls", bufs=2) as smalls,
            tc.tile_pool(name="singles", bufs=1) as singles,
            tc.tile_pool(name="psum", bufs=2, space="PSUM") as psum,
        ):
            wall = singles.tile([128, 4 * J * ROWS + J * P], f32r)
            nc.sync.dma_start(out=wall, in_=wall_d[:, :])
            w1c = wall[:P, 0 * J * ROWS : 1 * J * ROWS]
            w2c = wall[:P, 1 * J * ROWS : 2 * J * ROWS]
            w1p = wall[:P, 2 * J * ROWS : 3 * J * ROWS]
            w2p = wall[:P, 3 * J * ROWS : 4 * J * ROWS]
            wb = wall[:ROWS, 4 * J * ROWS :]
            warm = psum.tile([2, 2], f32, tag="s1", bufs=2)
            nc.tensor.matmul(warm, wall[:, 0:2], wall[:, 0:2], start=True, stop=True)

            # persistent squares tiles (parity double-buffer)
            sq0 = singles.tile([P, J, WC], f32r)
            sq1 = singles.tile([P, J, WC], f32r)
            sq_tiles = [sq0, sq1]

            def round_(dom, r, parity):
                if dom == "c":
                    W = WC
                    src_z, src_phi, src_prev, src_y = z_h, phic_h, xp1_h, yc_h
                    dst = xc_h
                    w1, w2 = w1c, w2c
                    wmain, raux = wc, rc
                else:
                    W = WP
                    src_z, src_phi, src_prev, src_y = z_h, phip_h, xc1_h, yp_h
                    dst = xp_h
                    w1, w2 = w1p, w2p
                    wmain, raux = wp, rp

                h0 = r * ROWS
                # partition = (c, hs) c-major with hs = row//J (8-row bands):
                # per c, the 4 hs bands are contiguous in DRAM, so the DGE
                # emits one ~40KB descriptor per channel (28 per DMA) and the
                # transfer spreads across all 16 SDMA engines.
                oap = [[H * W, NC], [J * W, HS], [1, J * W]]
                ooff = h0 * W

                zt = bigs.tile([P, J, W], f32, tag="z", name=f"z_{dom}{r}")
                if dom == "c":
                    zap = [[H * WC, NC], [J * WC, HS], [1, J * WC]]
                    nc.sync.dma_start(out=zt, in_=dram_ap(src_z, h0 * WC, zap))
                else:
                    # shifted view: w-runs of WP at row stride WC don't merge
                    # with the j dim -> one DMA per hs band (3 dims each)
                    for hs in range(HS):
                        zdst = bass.AP(
                            tensor=zt.tensor,
                            offset=zt.offset + hs * zt.ap[0][0],
                            ap=[[HS * zt.ap[0][0], NC], [zt.ap[1][0], J], [1, WP]],
                        )
                        nc.gpsimd.dma_start(
                            out=zdst,
                            in_=dram_ap(
                                src_z,
                                (h0 + hs * J) * WC,
                                [[H * WC + STEP, NC], [WC, J], [1, WP]],
                            ),
                        )
                pt = bigs.tile([P, J, W], f32, tag="phi", name=f"phi_{dom}{r}")
                nc.sync.dma_start(out=pt, in_=dram_ap(src_phi, ooff, oap))
                xt = bigs.tile([P, J, W], f32, tag="prev", name=f"prev_{dom}{r}")
                nc.scalar.dma_start(out=xt, in_=dram_ap(src_prev, ooff, oap))
                yt = smalls.tile([ROWS, W], f32, tag="y", name=f"y_{dom}{r}")
                nc.gpsimd.dma_start(out=yt, in_=src_y[h0 : h0 + ROWS, :])

                # a = z + raux * prev
                at = bigs.tile([P, J, W], f32, tag="a", name=f"a_{dom}{r}")
                nc.vector.scalar_tensor_tensor(
                    out=at, in0=xt, scalar=raux, in1=zt, op0=Alu.mult, op1=Alu.add
                )

                # squares on the scalar engine (row P stays 1.0)
                sq = sq_tiles[parity]
                nc.scalar.square(out=sq[:, :, :W], in_=pt)

                # products phi*a for the S1 reduction
                p1 = bigs.tile([P, J, W], f32r, tag="p1", name=f"p1_{dom}{r}")
                nc.vector.tensor_tensor(out=p1, in0=pt, in1=at, op=Alu.mult)

                # channel reductions on PE: S1 = -w * sum_c phi*a ; D = dofs + sum_c phi^2
                # each j-block has its own selector columns; the 8 matmuls
                # accumulate into one [32, W] PSUM tile at base partition 0.
                s1 = psum.tile([ROWS, W], f32, tag="s1", name=f"s1_{dom}{r}")
                s2 = psum.tile([ROWS, W], f32, tag="s2", name=f"s2_{dom}{r}")
                for j in range(J):
                    nc.tensor.matmul(
                        s1,
                        w1[:, ROWS * j : ROWS * (j + 1)],
                        p1[:, j, :],
                        start=(j == 0),
                        stop=(j == J - 1),
                    )
                for j in range(J):
                    nc.tensor.matmul(
                        s2,
                        w2[:, ROWS * j : ROWS * (j + 1)],
                        sq[:, j, :W],
                        start=(j == 0),
                        stop=(j == J - 1),
                    )

                # beta = (y + S1) / (S2 + dofs)
                dofs = dc0 if dom == "c" else dp0
                tt = smalls.tile([ROWS, W], f32, tag="tt", name=f"tt_{dom}{r}")
                nc.vector.tensor_tensor(out=tt, in0=yt, in1=s1, op=Alu.add)
                dd = smalls.tile([ROWS, W], f32, tag="dd", name=f"dd_{dom}{r}")
                nc.scalar.activation(
                    out=dd, in_=s2, func=mybir.ActivationFunctionType.Copy, bias=dofs
                )
                rcp = smalls.tile([ROWS, W], f32, tag="rcp", name=f"rcp_{dom}{r}")
                nc.vector.reciprocal_approx_fast(out=rcp, in_=dd)
                bt = smalls.tile([ROWS, W], f32r, tag="bt", name=f"bt_{dom}{r}")
                nc.vector.tensor_tensor(out=bt, in0=tt, in1=rcp, op=Alu.mult)

                # broadcast beta over channels (PE) and apply: out = w*a + phi*beta
                t2 = bigs.tile([P, J, W], f32, tag="t2", name=f"t2_{dom}{r}")
                for g in range(J // 2):
                    bj = psum.tile(
                        [P, 2, 512], f32, tag="bj", bufs=2, name=f"bj_{dom}{r}_{g}"
                    )
                    for jj in range(2):
                        nc.tensor.matmul(
                            bj[:, jj, :W],
                            wb[:, P * (2 * g + jj) : P * (2 * g + jj + 1)],
                            bt[:, :],
                            start=True,
                            stop=True,
                        )
                    nc.vector.tensor_tensor(
                        out=t2[:, 2 * g : 2 * g + 2, :],
                        in0=pt[:, 2 * g : 2 * g + 2, :],
                        in1=bj[:, :, :W],
                        op=Alu.mult,
                    )

                ot = bigs.tile([P, J, W], f32, tag="out", name=f"out_{dom}{r}")
                nc.vector.scalar_tensor_tensor(
                    out=ot, in0=at, scalar=wmain, in1=t2, op0=Alu.mult, op1=Alu.add
                )
                nc.gpsimd.dma_start(out=dram_ap(dst, ooff, oap), in_=ot)

            i = 0
            for dom in ("c", "p"):
                for r in range(R):
                    round_(dom, r, i % 2)
                    i += 1

    nc.compile()
    return nc


def kernel(z, yc, phi_c, yp, phi_p, xc_k_1, xp_k_1, mu, mu_c, mu_p):
    global LAST_RESULTS
    from concourse.bass_utils import run_bass_kernel_spmd

    mu_ = float(np.asarray(mu).reshape(-1)[0])
    mu_c_ = float(np.asarray(mu_c).reshape(-1)[0])
    mu_p_ = float(np.asarray(mu_p).reshape(-1)[0])

    key = (mu_, mu_c_, mu_p_)
    nc = _PROGRAM_CACHE.get(key)
    if nc is None:
        nc = _build(mu_, mu_c_, mu_p_)
        _PROGRAM_CACHE[key] = nc

    z = np.ascontiguousarray(np.asarray(z), np.float32)
    yc = np.ascontiguousarray(np.asarray(yc), np.float32)
    phi_c = np.ascontiguousarray(np.asarray(phi_c), np.float32)
    yp = np.ascontiguousarray(np.asarray(yp), np.float32)
    phi_p = np.ascontiguousarray(np.asarray(phi_p), np.float32)
    xc_k_1 = np.ascontiguousarray(np.asarray(xc_k_1), np.float32)
    xp_k_1 = np.ascontiguousarray(np.asarray(xp_k_1), np.float32)

    in_maps = [
        {
            "z": z[b],
            "phi_c": phi_c[b],
            "xp_k_1": xp_k_1[b],
            "yc": yc[b],
            "phi_p": phi_p[b],
            "xc_k_1": xc_k_1[b],
            "yp": yp[b],
        }
        for b in range(B)
    ]

    res = run_bass_kernel_spmd(nc, in_maps, core_ids=list(range(B)), trace=TRACE)
    LAST_RESULTS = res
    xc = np.stack([res.results[b]["xc"] for b in range(B)])
    xp = np.stack([res.results[b]["xp"] for b in range(B)])
    return xc, xp
